# revision 1
# baseline (speedup 1.0000x reference)
"""Raw-bacc (no TileContext) kernel: hand-scheduled engines + semaphores.

Math: result[i,j] = sum_k relu((a@feats.T)[i,k]) * ((b@feats.T)[j,k] <= 0)
    = relu(a @ feats.T) @ ((b @ feats.T) <= 0).T

Sharding: 2x4 grid over (a-rows, b-rows); each of 8 cores computes a
[512, 256] tile. Host pre-transposes/packs inputs (fp32 on-chip transposes
are expensive; DMA transpose is 16-bit only):
  bfT  [128, 512] f32  = [feats.T | b_shard.T]   (sign-critical, fp32)
  afTb [128, 768] bf16 = [feats.T | a_shard.T]   (magnitude-only, bf16)

Precision: the b-projection decides mask signs -> fp32 (a flipped sign admits
a full-sized relu(afk) term). a-projection + final matmul are bf16 (1 PE
cycle/row vs 4 for fp32). Optional bf16 output (host upcasts to f32).

Engine plan (PSUM readers can only be DVE/ACT; GpSimd must not touch PSUM):
  SP  : input DMAs, output DMAs for row-blocks 0,1
  ACT : LUT warmup during load wait, relu(pa0/pa1), copy po2, out DMA 2+3
  PE  : warmup matmuls (p-state), b-proj fp32, a-proj bf16, finals bf16
  DVE : is_le(pb0), is_le(pb1), copies po0/po1
  POOL: memsets, final wait + dma_reset + sem_clear (epilogue)

PSUM banks (8): pb0 pb1 pa0 pa1 po0..po3. Warmup matmuls write po3[:16,:16],
overwritten by ic3's start=True matmul. No same-bank PE-write + DVE/ACT-read
is ever concurrent (enforced by the semaphore edges).

repeat=R iterates the whole body R times inside one NEFF (cross-iteration
WAR hazards guarded by the same counting sems); used to measure steady-state
per-iteration HW time via wall-clock deltas. The submitted kernel uses R=1.
"""

import numpy as np
import ml_dtypes

Na, Nb, K, D = 1024, 1024, 256, 128
MA, MB = 2, 4
NAS, NBS = Na // MA, Nb // MB
P = 128
N_CORES = 8
KC = K // P
IC = NAS // P
N_WARM = 16
OUT_BF16 = True

_COMPILED = {}


def _build(repeat=1, epilogue_clear=True, out_bf16=OUT_BF16):
    from concourse import bacc, mybir

    f32 = mybir.dt.float32
    bf16 = mybir.dt.bfloat16
    ops = mybir.AluOpType
    AF = mybir.ActivationFunctionType
    odt = bf16 if out_bf16 else f32

    nc = bacc.Bacc("TRN2", target_bir_lowering=False, debug=False,
                   num_devices=N_CORES)

    # Drop the framework preamble (4 const-AP memsets + all-engine startup
    # barrier, ~620ns): this kernel reads none of the const APs and never
    # uses all_engine_barrier.
    _entry = nc.m.functions[0].blocks[0]
    for _inst in [i for i in _entry.instructions
                  if type(i).__name__ in ("InstMemset", "InstDrain",
                                          "InstEventSemaphore")]:
        _entry.instructions.remove(_inst)

    bfT_d = nc.dram_tensor("bfT", [P, K + NBS], f32, kind="ExternalInput").ap()
    afTb_d = nc.dram_tensor("afTb", [P, K + NAS], bf16,
                            kind="ExternalInput").ap()
    out_d = nc.dram_tensor("out", [NAS, NBS], odt, kind="ExternalOutput").ap()
    out_3d = out_d.rearrange("(po pi) f -> pi po f", pi=P)

    bfT = nc.alloc_sbuf_tensor("bfT_sb", [P, K + NBS], f32).ap()
    afTb = nc.alloc_sbuf_tensor("afTb_sb", [P, K + NAS], bf16).ap()
    QT = [nc.alloc_sbuf_tensor(f"qt{c}", [P, NBS], bf16).ap() for c in range(KC)]
    PT = [nc.alloc_sbuf_tensor(f"pt{c}", [P, NAS], bf16).ap() for c in range(KC)]
    out_sb = nc.alloc_sbuf_tensor("out_sb", [P, IC, NBS], odt).ap()
    warm_sb = nc.alloc_sbuf_tensor("warm_sb", [P, 128], bf16).ap()
    zbias = nc.alloc_sbuf_tensor("zbias", [P, 1], f32).ap()
    zs1 = nc.alloc_sbuf_tensor("zs1", [P, 1], f32).ap()
    zs2 = nc.alloc_sbuf_tensor("zs2", [P, 1], f32).ap()

    pb = [nc.alloc_psum_tensor(f"pb{c}", [P, NBS], f32).ap() for c in range(KC)]
    pa = [nc.alloc_psum_tensor(f"pa{c}", [P, NAS], f32).ap() for c in range(KC)]
    po = [nc.alloc_psum_tensor(f"po{i}", [P, NBS], f32).ap() for i in range(IC)]

    sems = {}
    for s in ["warm", "z", "bfT", "afTb", "pb0", "pb1", "pa0", "pa1",
              "q0", "q1", "p0", "p1", "po", "cpd", "cpa", "c0",
              "o0", "o1", "oa", "disp"]:
        sems[s] = nc.alloc_semaphore(f"s_{s}")

    fT0 = bfT[:, :P]
    fT1 = bfT[:, P:K]
    bT = bfT[:, K:]
    fTb = afTb[:, :K]
    aT = afTb[:, K:]

    # ---- POOL prologue ----
    nc.gpsimd.memset(warm_sb[:], 0.0).then_inc(sems["warm"], 1)
    nc.gpsimd.memset(zbias[:], 0.0).then_inc(sems["z"], 1)

    # ---- ACT LUT warmup (Copy + Relu tables) during the load wait ----
    nc.scalar.activation(zs1[:], zbias[:], AF.Copy)._wait_ge(sems["z"], 1)
    nc.scalar.activation(zs2[:], zbias[:], AF.Relu, bias=zbias[:])

    # ---- PE warmup: keep the PE continuously busy until the first real
    # matmul's data lands, so the p-state ramp (>3us sustained) completes and
    # the projections run at peak clock instead of mid.
    for i in range(N_WARM):
        mm = nc.tensor.matmul(po[3][:16, :128], warm_sb[:, :16], warm_sb[:],
                              skip_group_check=True)
        if i == 0:
            mm._wait_ge(sems["warm"], 1)

    for r in range(repeat):
        # ---- SP: input DMAs (WAR: prev iteration's projections done) ----
        dma = nc.sync.dma_start(bfT[:], bfT_d[:])
        if r > 0:
            dma._wait_ge(sems["pb1"], r)
        dma.then_inc(sems["bfT"], 16)
        dma = nc.sync.dma_start(afTb[:], afTb_d[:])
        if r > 0:
            dma._wait_ge(sems["pa1"], r)
        dma.then_inc(sems["afTb"], 16)

        # ---- PE: projections pb0, pa0, pa1, pb1 (pb1 last: its 853ns
        # overlaps both relus so the c1 round isn't gated on them) ----
        if r > 0:
            nc.tensor.wait_ge(sems["q0"], r)      # WAR: is_le0(r-1) read pb0
        nc.tensor.matmul(pb[0][:], fT0[:], bT[:])._wait_ge(
            sems["bfT"], 16 * (r + 1)).then_inc(sems["pb0"], 1)
        if r > 0:
            nc.tensor.wait_ge(sems["p0"], r)
        nc.tensor.matmul(pa[0][:], fTb[:, :P], aT[:])._wait_ge(
            sems["afTb"], 16 * (r + 1)).then_inc(sems["pa0"], 1)
        if r > 0:
            nc.tensor.wait_ge(sems["p1"], r)
        nc.tensor.matmul(pa[1][:], fTb[:, P:], aT[:]).then_inc(sems["pa1"], 1)
        if r > 0:
            nc.tensor.wait_ge(sems["q1"], r)
        nc.tensor.matmul(pb[1][:], fT1[:], bT[:])._wait_ge(
            sems["bfT"], 16 * (r + 1)).then_inc(sems["pb1"], 1)

        # ---- DVE: is_le0, is_le1 (masks); ACT does both relus in parallel -
        if r > 0:
            nc.vector.wait_ge(sems["c0"], r)      # WAR: c0 mms(r-1) read QT0
        nc.vector.tensor_scalar(out=QT[0][:], in0=pb[0][:], scalar1=0.0,
                                scalar2=None, op0=ops.is_le)._wait_ge(
            sems["pb0"], r + 1).then_inc(sems["q0"], 1)
        if r > 0:
            nc.vector.wait_ge(sems["po"], 4 * r)  # WAR: c1 mms(r-1) read QT1
        nc.vector.tensor_scalar(out=QT[1][:], in0=pb[1][:], scalar1=0.0,
                                scalar2=None, op0=ops.is_le)._wait_ge(
            sems["pb1"], r + 1).then_inc(sems["q1"], 1)

        # ---- ACT: relu0, relu1 ----
        if r > 0:
            nc.scalar.wait_ge(sems["c0"], r)      # WAR: c0 mms(r-1) read PT0
        nc.scalar.activation(PT[0][:], pa[0][:], AF.Relu,
                             bias=zbias[:])._wait_ge(
            sems["pa0"], r + 1).then_inc(sems["p0"], 1)
        if r > 0:
            nc.scalar.wait_ge(sems["po"], 4 * r)  # WAR: c1 mms(r-1) read PT1
        nc.scalar.activation(PT[1][:], pa[1][:], AF.Relu,
                             bias=zbias[:])._wait_ge(
            sems["pa1"], r + 1).then_inc(sems["p1"], 1)

        # ---- PE: finals, c=0 round then c=1 round ----
        for ic in range(IC):
            if ic == 0:
                nc.tensor.wait_ge(sems["q0"], r + 1)
            if r > 0:
                # WAR: copy of this po from the previous iteration done
                cs, cv = {0: ("cpd", 3 * r - 1), 1: ("cpd", 3 * r),
                          2: ("cpa", r), 3: ("cpd", 3 * r - 2)}[ic]
                nc.tensor.wait_ge(sems[cs], cv)
            mm = nc.tensor.matmul(po[ic][:], PT[0][:, ic * P:(ic + 1) * P],
                                  QT[0][:], start=True, stop=False,
                                  skip_group_check=True)
            if ic == 0:
                mm._wait_ge(sems["p0"], r + 1)
            if ic == IC - 1:
                mm.then_inc(sems["c0"], 1)
        for n, ic in enumerate((2, 3, 0, 1)):
            if n == 0:
                nc.tensor.wait_ge(sems["q1"], r + 1)
            mm = nc.tensor.matmul(po[ic][:], PT[1][:, ic * P:(ic + 1) * P],
                                  QT[1][:], start=False, stop=True,
                                  skip_group_check=True)
            if n == 0:
                mm._wait_ge(sems["p1"], r + 1)
            mm.then_inc(sems["po"], 1)

        # ---- DVE: copy3 first (parallel with ACT's copy2, so the tail
        # DMA's inputs finish ~300ns earlier), then copies po0/po1 ----
        if r > 0:
            nc.vector.wait_ge(sems["oa"], 16 * r)   # WAR: out DMA(r-1) done
        nc.vector.tensor_copy(out=out_sb[:, 3, :], in_=po[3][:])._wait_ge(
            sems["po"], 4 * r + 2).then_inc(sems["cpd"], 1)
        if r > 0:
            nc.vector.wait_ge(sems["o0"], 16 * r)
        nc.vector.tensor_copy(out=out_sb[:, 0, :], in_=po[0][:])._wait_ge(
            sems["po"], 4 * r + 3).then_inc(sems["cpd"], 1)
        if r > 0:
            nc.vector.wait_ge(sems["o1"], 16 * r)
        nc.vector.tensor_copy(out=out_sb[:, 1, :], in_=po[1][:])._wait_ge(
            sems["po"], 4 * r + 4).then_inc(sems["cpd"], 1)


        # ---- ACT: copies po2/po3, merged out DMA for blocks 2+3 ----
        if r > 0:
            nc.scalar.wait_ge(sems["oa"], 16 * r)   # WAR: out DMA(r-1) done
        nc.scalar.activation(out_sb[:, 2, :], po[2][:], AF.Copy)._wait_ge(
            sems["po"], 4 * r + 1).then_inc(sems["cpa"], 1)
        nc.scalar.wait_ge(sems["cpd"], 3 * r + 1)   # copy3 (on DVE) done
        dma = nc.scalar.dma_start(out_3d[:, 2:4, :], out_sb[:, 2:4, :])
        dma._wait_ge(sems["cpa"], r + 1).then_inc(sems["oa"], 16)
        if r == repeat - 1:
            nc.scalar.sem_inc(sems["disp"], 1)   # out DMAs enqueued (ACT)

        # ---- SP: out DMAs for blocks 0/1 ----
        nc.sync.dma_start(out_3d[:, 0, :], out_sb[:, 0, :])._wait_ge(
            sems["cpd"], 3 * r + 2).then_inc(sems["o0"], 16)
        nc.sync.dma_start(out_3d[:, 1, :], out_sb[:, 1, :])._wait_ge(
            sems["cpd"], 3 * r + 3).then_inc(sems["o1"], 16)
        if r == repeat - 1:
            nc.sync.sem_inc(sems["disp"], 1)     # out DMAs enqueued (SP)

    # ---- POOL epilogue: hold NEFF open until outputs land, reset sems ----
    # s_disp >= 2 => every out DMA of the last iteration is ENQUEUED (engine
    # in-order dispatch); the DRAIN then waits for the DMA queues to finish
    # directly, skipping the ~0.9us DMA-completion semaphore propagation.
    nc.gpsimd.wait_ge(sems["disp"], 2)
    sem_nums = sorted(s.num for s in sems.values())
    lo, hi = sem_nums[0], sem_nums[-1]
    assert sem_nums == list(range(lo, hi + 1)), sem_nums
    nc.gpsimd.dma_reset(range(lo, hi + 1))
    if epilogue_clear:
        nc.gpsimd.sem_clear(range(lo, hi + 1))

    nc.compile()
    return nc


def _get_compiled(repeat=1):
    if repeat not in _COMPILED:
        _COMPILED[repeat] = _build(repeat=repeat)
    return _COMPILED[repeat]


def _make_in_maps(a, b, feats):
    a = np.asarray(a, dtype=np.float32)
    b = np.asarray(b, dtype=np.float32)
    feats = np.asarray(feats, dtype=np.float32)

    fT = np.ascontiguousarray(feats.T)
    fT_bf = fT.astype(ml_dtypes.bfloat16)
    bfT = [np.ascontiguousarray(
        np.concatenate([fT, b[j * NBS:(j + 1) * NBS].T], axis=1))
        for j in range(MB)]
    afTb = [np.ascontiguousarray(np.concatenate(
        [fT_bf, a[i * NAS:(i + 1) * NAS].T.astype(ml_dtypes.bfloat16)],
        axis=1)) for i in range(MA)]
    return [{"bfT": bfT[c % MB], "afTb": afTb[c // MB]} for c in range(N_CORES)]


def _gather(results):
    out = np.empty((Na, Nb), dtype=np.float32)
    for core in range(N_CORES):
        ai, bi = core // MB, core % MB
        out[ai * NAS:(ai + 1) * NAS, bi * NBS:(bi + 1) * NBS] = \
            results[core]["out"].astype(np.float32)
    return out


def run(a, b, feats, trace=False, repeat=1):
    from concourse.bass_utils import run_bass_kernel_spmd

    nc = _get_compiled(repeat)
    in_maps = _make_in_maps(a, b, feats)
    res = run_bass_kernel_spmd(nc, in_maps, list(range(N_CORES)), trace=trace)
    return _gather(res.results), res


def kernel(a, b, feats):
    out, _ = run(a, b, feats)
    return out

